# revision 1
# baseline (speedup 1.0000x reference)
"""Bidirectional minGRU (nn_MinGRU2) Trainium2 Bass kernel.

Full input x: [16, 512, 4096] f32. Channel layout per batch:
    0:128    forward h        128:256  forward g
    256:384  backward h       384:512  backward g
Output [16, 256, 4096]: out[:, 0:128] = forward minGRU, out[:, 128:256] =
backward minGRU (scanned right-to-left over L).

The log-space reference reduces to the direct linear recurrence per
(b, channel) lane:
    sig  = sigmoid(g);  coef = sigmoid(-g);  v = h * sig
    y[t] = coef[t] * y[t-1] + v[t]
which maps to one DVE tensor_tensor_scan per [128-lane, L-chunk] tile, with
ACT computing both sigmoids and DVE the multiply. The backward direction
runs the same scan through reversed (negative-stride) access patterns, so
no explicit flip pass is needed.

All HBM traffic and SBUF tiles are fp16: the host casts x to fp16 before
upload and the output back to f32 after download. This halves DMA bytes
(the dominant cost; HBM-per-core is the roofline) and lets the DVE multiply
run in 2x_1p mode; the scan itself carries fp32 state internally regardless
of operand dtype. Loads are issued as 4 smaller DMAs per tile (load_split):
finer transfer granularity interleaves loads/stores better across the SDMA
engines and measured consistently faster than one 1 MB DMA per tile.

Sharding: fully data-parallel over batch — 16 batches / 8 cores = 2 per
core; every (b, lane) recurrence is independent and L stays contiguous.
"""
import numpy as np

import concourse.bacc as bacc
import concourse.mybir as mybir
import concourse.tile as tile
from concourse.bass_utils import run_bass_kernel_spmd

B, H, L = 16, 512, 4096
N_CORES = 8
B_PC = B // N_CORES  # batches per core

P = 128
F16 = mybir.dt.float16
MULT = mybir.AluOpType.mult
ADD = mybir.AluOpType.add
SIGMOID = mybir.ActivationFunctionType.Sigmoid

CHUNK = 2048
BUFS = 3
IN_BUFS = 4
OUT_BUFS = 6


def _emit(tc: tile.TileContext, x, out, chunk=CHUNK, bufs=BUFS, out_bufs=OUT_BUFS,
          store_eng=0, in_bufs=IN_BUFS, first=0, batch_store=False,
          coef_on="act", mult_on="dve", load_split=4, load_hg=False,
          load_eng="sync", scan_split=1, store_split=1, pools=None):
    nc = tc.nc
    # chunk schedule over L; `first` splits a smaller leading chunk off the
    # first full chunk so compute/stores start earlier (shorter pipeline fill)
    sizes = [chunk] * (L // chunk)
    if first:
        sizes = [first, chunk - first] + sizes[1:]
    # streams: (batch, direction); direction 0 = forward, 1 = backward
    streams = [(b, d) for b in range(B_PC) for d in (0, 1)]
    carries = {s: None for s in streams}
    store = (nc.gpsimd, nc.scalar, nc.sync)[store_eng]

    # out tiles live across a chunk boundary (the next chunk's scan reads the
    # carry column), so with S streams in flight up to S+1 must coexist —
    # fewer slots can cycle with engine program order and deadlock.
    import contextlib
    if pools is not None:
        # pools shared across repeats: slots cycle continuously, no per-call
        # scope setup/teardown between repeats
        cm_io, cm_mid, cm_op = (contextlib.nullcontext(p) for p in pools)
    else:
        cm_io = tc.tile_pool(name="io", bufs=in_bufs)
        cm_mid = tc.tile_pool(name="mid", bufs=bufs)
        cm_op = tc.tile_pool(name="op", bufs=out_bufs)
    with cm_io as io, cm_mid as mid, cm_op as op:
        starts = [sum(sizes[:i]) for i in range(len(sizes))]
        # batch_store: both directions of a batch accumulate into one
        # [128, 2, L] tile, stored as a single 2 MB DMA per batch. Only
        # valid when the whole L fits one chunk per stream.
        bstore = {}
        if batch_store:
            assert len(sizes) == 1
            for b in range(B_PC):
                bstore[b] = op.tile([P, 2, L], F16, tag=f"bs{b}", name=f"bstore{b}")
        for k, (k0, chunk) in enumerate(zip(starts, sizes)):
            for (b, d) in streams:
                # forward walks L ascending, backward descending
                l0 = k0 if d == 0 else L - k0 - chunk
                sl = slice(l0, l0 + chunk)

                # one DMA: the stream's h- and g-quarters (256 adjacent
                # channels) -> [128 part, 2, chunk]
                in_t = io.tile([P, 2, chunk], F16, tag="in")
                src = x[b, d * 256:(d + 1) * 256, sl]
                src = src.rearrange("(q p) l -> p q l", p=P)
                sidx0 = 2 * b + d
                ld = {"sync": nc.sync, "scalar": nc.scalar,
                      "mix": (nc.sync, nc.scalar)[sidx0 % 2],
                      "mix3": (nc.sync, nc.scalar, nc.vector)[sidx0 % 3]}[load_eng]
                if load_hg:
                    # separate h and g transfers: one contiguous HBM row per
                    # partition each, and the g half (which feeds the longer
                    # ACT->DVE chain) can land first; composes with load_split
                    step = chunk // load_split
                    for j in range(load_split):
                        js = slice(j * step, (j + 1) * step)
                        ld.dma_start(out=in_t[:, 1, js], in_=src[:, 1, js])
                    for j in range(load_split):
                        js = slice(j * step, (j + 1) * step)
                        ld.dma_start(out=in_t[:, 0, js], in_=src[:, 0, js])
                elif load_split == 1:
                    ld.dma_start(out=in_t, in_=src)
                else:
                    # keep DMA granularity below the compute-chunk size:
                    # several smaller transfers pipeline better while the
                    # compute ops still run at full-chunk width
                    step = chunk // load_split
                    for j in range(load_split):
                        js = slice(j * step, (j + 1) * step)
                        ld.dma_start(
                            out=in_t[:, :, js], in_=src[:, :, js])

                h_ap = in_t[:, 0, :]
                g_ap = in_t[:, 1, :]

                sidx = 2 * b + d
                sig = mid.tile([P, chunk], F16, tag="sig")
                nc.scalar.activation(sig, g_ap, SIGMOID)
                coef = mid.tile([P, chunk], F16, tag="coef")
                # coef = sigmoid(-g) = 1 - sig: second ACT pass, or a DVE
                # tensor_scalar (runs in 4x mode on f16 — 4 elem/lane/cycle)
                use_dve_coef = coef_on == "dve" or (coef_on == "mix" and sidx % 2)
                if use_dve_coef:
                    nc.vector.tensor_scalar(
                        out=coef, in0=sig, scalar1=-1.0, scalar2=1.0,
                        op0=MULT, op1=ADD)
                else:
                    nc.scalar.activation(coef, g_ap, SIGMOID, scale=-1.0)
                v = mid.tile([P, chunk], F16, tag="v")
                use_pool_mult = mult_on == "pool" or (mult_on == "mix" and sidx % 2)
                mult_eng = nc.gpsimd if use_pool_mult else nc.vector
                mult_eng.tensor_tensor(out=v, in0=h_ap, in1=sig, op=MULT)

                if batch_store:
                    out_t = bstore[b][:, d, :]
                else:
                    out_t = op.tile([P, chunk], F16, tag="out")
                # scan_split: run the (inherently serial, 1 col/cycle) scan
                # in sub-pieces chained through carries, so each piece's
                # store fires while the next piece scans
                sstep = chunk // scan_split
                for j in range(scan_split):
                    jj = j if d == 0 else scan_split - 1 - j
                    ssl = slice(jj * sstep, (jj + 1) * sstep)
                    init = carries[(b, d)]
                    if init is None:
                        init = 0.0
                    if d == 0:
                        nc.vector.tensor_tensor_scan(
                            out=out_t[:, ssl], data0=coef[:, ssl],
                            data1=v[:, ssl], initial=init,
                            op0=MULT, op1=ADD)
                        carries[(b, d)] = out_t[:, ssl.stop - 1:ssl.stop]
                    else:
                        nc.vector.tensor_tensor_scan(
                            out=out_t[:, ssl][:, ::-1],
                            data0=coef[:, ssl][:, ::-1],
                            data1=v[:, ssl][:, ::-1], initial=init,
                            op0=MULT, op1=ADD)
                        carries[(b, d)] = out_t[:, ssl.start:ssl.start + 1]

                    if not batch_store:
                        # store on SWDGE (gpsimd) so store triggers (which
                        # wait on the scan) don't block load issue on SP;
                        # store_split issues the completed piece as several
                        # smaller DMAs for finer SDMA interleaving
                        tstep = (ssl.stop - ssl.start) // store_split
                        for m in range(store_split):
                            ts = slice(ssl.start + m * tstep,
                                       ssl.start + (m + 1) * tstep)
                            store.dma_start(
                                out=out[b, d * P:(d + 1) * P,
                                        l0 + ts.start:l0 + ts.stop],
                                in_=out_t[:, ts])
        if batch_store:
            for b in range(B_PC):
                store.dma_start(
                    out=out[b].rearrange("(q p) l -> p q l", p=P),
                    in_=bstore[b])


_NC_CACHE = {}


def build(n_repeat=1, share_pools=True, **emit_kwargs):
    key = (n_repeat, share_pools, tuple(sorted(emit_kwargs.items())))
    if key not in _NC_CACHE:
        nc = bacc.Bacc("TRN2", target_bir_lowering=False, debug=False)
        x = nc.dram_tensor("x", [B_PC, H, L], F16, kind="ExternalInput")
        out = nc.dram_tensor("out", [B_PC, H // 2, L], F16, kind="ExternalOutput")
        with tile.TileContext(nc) as tc:
            if share_pools and n_repeat > 1:
                with tc.tile_pool(name="io", bufs=emit_kwargs.get("in_bufs", IN_BUFS)) as io, \
                     tc.tile_pool(name="mid", bufs=emit_kwargs.get("bufs", BUFS)) as mid, \
                     tc.tile_pool(name="op", bufs=emit_kwargs.get("out_bufs", OUT_BUFS)) as op:
                    for _ in range(n_repeat):
                        _emit(tc, x.ap(), out.ap(), pools=(io, mid, op),
                              **emit_kwargs)
            else:
                for _ in range(n_repeat):
                    _emit(tc, x.ap(), out.ap(), **emit_kwargs)
        nc.compile()
        _NC_CACHE[key] = nc
    return _NC_CACHE[key]


def kernel(x: np.ndarray):
    assert x.shape == (B, H, L) and x.dtype == np.float32
    nc = build()
    x16 = x.astype(np.float16)
    in_maps = [
        {"x": np.ascontiguousarray(x16[i * B_PC:(i + 1) * B_PC])}
        for i in range(N_CORES)
    ]
    res = run_bass_kernel_spmd(nc, in_maps, core_ids=list(range(N_CORES)))
    return np.concatenate(
        [r["out"] for r in res.results], axis=0).astype(np.float32)

